# revision 19
# baseline (speedup 1.0000x reference)
"""GCNConv (X @ W sparse-aggregated) Trainium2 kernel, 8-core SPMD.

Math: out = segment_sum(edge_val * (X@W)[edge_col], edge_row) + bias
Reformulated via associativity:  out = H @ W + bias, where
    H = segment_sum(edge_val * X[edge_col], edge_row)          # [N, F]

Sharding: destination nodes are sorted by in-degree and dealt round-robin
across the 8 cores, so every core gets an identical per-tile "round"
structure (same compiled program on all cores).  The host pre-gathers
X[edge_col] into a round-major, partition-contiguous layout so the device
only does large sequential DMA; the device then:
  1. scales each gathered row by its edge value (DVE/GPSIMD multiply; the
     edge values are pre-replicated 8x on host so the innermost AP dim is
     unit-stride 2-byte -> DVE 2x perf mode)
  2. scatter-reduces rounds into H.T tiles with PE matmuls against an
     identity (PSUM accumulation: lhsT=scaled rows, rhs=I)
  3. computes out.T = W.T @ H.T with a second PE matmul, adds bias during
     the PSUM->SBUF copy (ACT), and streams out.T to HBM.
The host un-permutes/transposes the per-core outputs into the full result.

Raw Bass (no Tile framework): this walrus build allows only ONE attached
sync-wait per compute instruction; standalone wait_ge sequencer ops have no
such limit and the pipeline is static, so explicit counters work.
DMA completion semaphores are per-buffer-slot: a slot's wait target always
equals the total count of DMAs ever issued on that semaphore at wait time,
so partial-completion skew across the 16 SDMA engines cannot fire it early.
"""

import numpy as np

N_NODES = 50000
N_EDGES = 800000
F = 128
P = 128
N_CORES = 8
SPAN = P * N_CORES               # 1024 degree-sorted nodes per tile-span
N_TILES = (N_NODES + SPAN - 1) // SPAN      # 49
NPOS = N_TILES * SPAN            # 50176 padded positions
SLOTS = N_TILES * P              # 6272 node slots per core
VDUP = 4                         # host-side replication of edge values

_KERNEL_CACHE = {}


def _scale_engine(k):
    """Scale engine per tile. All on DVE: its 2x perf mode matches DMA and PE
    rates (~90ns per round each), and concurrent GPSIMD tensor ops contend
    for the same SBUF ports, slowing DVE to 1x."""
    return "v"


def _build_nc(R):
    from contextlib import ExitStack

    import concourse.bass as bass
    import concourse.mybir as mybir

    f16 = mybir.dt.float16
    f32 = mybir.dt.float32

    NT = N_TILES
    B = int(np.sum(R))
    boffs = np.zeros(NT, dtype=np.int64)
    boffs[1:] = np.cumsum(R)[:-1]

    # group structure: 4 tiles per group (one DMA slab + one N=512 GEMM2)
    groups = []  # (first_tile, gsize)
    kk = 0
    while kk < NT:
        gs = min(4, NT - kk)
        groups.append((kk, gs))
        kk += gs
    NG = len(groups)
    group_of = np.zeros(NT, dtype=np.int64)
    for gi, (k0, gs) in enumerate(groups):
        group_of[k0 : k0 + gs] = gi
    # rounds per group and max (for slab sizing)
    gR = [int(sum(R[k0 : k0 + gs])) for (k0, gs) in groups]
    GRmax = max(gR)
    g_boff = [int(boffs[k0]) for (k0, _gs) in groups]

    # split each group's slab load in two (by tiles) for finer pipelining
    g_halves = []          # per group: list of (round_start, round_end) in slab
    tile_xs_sem = {}       # tile -> (sem index, completed-load count on it)
    slot_loads = [0, 0, 0, 0, 0, 0]
    # split the final single-tile group's work in two parts (shorter tail)
    k_split = groups[-1][0] if groups[-1][1] == 1 else None
    Ra = int(R[k_split]) // 2 if k_split is not None else 0
    split_xs = []           # (sem, cnt) for each part's half-load
    for gi, (k0, gs) in enumerate(groups):
        if gs == 1 and k0 == k_split:
            halves = [(0, Ra), (Ra, gR[gi])]
            g_halves.append(halves)
            for hi, (ra, rb) in enumerate(halves):
                si = 2 * (gi % 3) + hi
                slot_loads[si] += 1
                split_xs.append((si, slot_loads[si]))
            tile_xs_sem[k0] = split_xs[0]
            continue
        h1 = (gs + 1) // 2
        cut = int(boffs[k0 + h1 - 1] + R[k0 + h1 - 1] - g_boff[gi]) if h1 > 0 else 0
        halves = [(0, cut)]
        if cut < gR[gi]:
            halves.append((cut, gR[gi]))
        g_halves.append(halves)
        for hi, (ra, rb) in enumerate(halves):
            si = 2 * (gi % 3) + hi
            slot_loads[si] += 1
            lo = k0 if hi == 0 else k0 + h1
            hi_t = (k0 + h1 - 1) if hi == 0 else (k0 + gs - 1)
            for k in range(lo, hi_t + 1):
                tile_xs_sem[k] = (si, slot_loads[si])

    eng_of = [_scale_engine(k) for k in range(NT)]
    cnt_after = {"v": np.zeros(NT, dtype=np.int64), "p": np.zeros(NT, dtype=np.int64)}
    cv = cp = 0
    for k in range(NT):
        if eng_of[k] == "v":
            cv += 1
        else:
            cp += 1
        cnt_after["v"][k] = cv
        cnt_after["p"][k] = cp

    nc = bass.Bass(target_bir_lowering=False, debug=False)

    XRT = nc.declare_dram_parameter("xrt", [P, B, F], f16, isOutput=False)
    VEX = nc.declare_dram_parameter("vex", [P, B, VDUP], f16, isOutput=False)
    WP = nc.declare_dram_parameter("w", [F, F], f16, isOutput=False)
    BIASP = nc.declare_dram_parameter("bias", [F, 1], f32, isOutput=False)
    IDP = nc.declare_dram_parameter("ident", [P, P], f16, isOutput=False)
    OUT = nc.declare_dram_parameter("out", [F, SLOTS], f16, isOutput=True)

    with ExitStack() as ctx:
        ident = ctx.enter_context(nc.sbuf_tensor("identsb", [P, P], f16))
        wsb = ctx.enter_context(nc.sbuf_tensor("wsb", [F, F], f16))
        vex = ctx.enter_context(nc.sbuf_tensor("vexsb", [P, B, VDUP], f16))
        bias = ctx.enter_context(nc.sbuf_tensor("biassb", [F, 1], f32))
        xs = [ctx.enter_context(nc.sbuf_tensor(f"xs{i}", [P, GRmax, F], f16)) for i in range(3)]
        sc = [ctx.enter_context(nc.sbuf_tensor(f"sc{i}", [P, GRmax, F], f16)) for i in range(2)]
        ht = [ctx.enter_context(nc.sbuf_tensor(f"ht{i}", [P, 4 * P], f16)) for i in range(2)]
        osb = [ctx.enter_context(nc.sbuf_tensor(f"osb{i}", [P, 4 * P], f16)) for i in range(2)]
        pha = [ctx.enter_context(nc.psum_tensor(f"pha{i}", [P, 512], f32)) for i in range(3)]
        phb = [ctx.enter_context(nc.psum_tensor(f"phb{i}", [P, 512], f32)) for i in range(2)]
        phw = ctx.enter_context(nc.psum_tensor("phw", [P, 512], f32))

        s_cst = ctx.enter_context(nc.semaphore("s_cst"))
        s_xs = [ctx.enter_context(nc.semaphore(f"s_xs{i}")) for i in range(6)]
        s_scv = ctx.enter_context(nc.semaphore("s_scv"))
        s_scp = ctx.enter_context(nc.semaphore("s_scp"))
        s_peA = ctx.enter_context(nc.semaphore("s_peA"))
        s_peB = ctx.enter_context(nc.semaphore("s_peB"))
        s_acth = ctx.enter_context(nc.semaphore("s_acth"))
        s_acto = ctx.enter_context(nc.semaphore("s_acto"))
        s_odma = [ctx.enter_context(nc.semaphore(f"s_odma{i}")) for i in range(2)]
        all_sems = [s_cst, *s_xs, s_scv, s_scp, s_peA, s_peB, s_acth, s_acto, *s_odma]
        s_sem = {"v": s_scv, "p": s_scp}

        for s in all_sems:
            nc.sync.sem_clear(s)
        nc.all_engine_barrier()

        def scale_in_aps(k, r0=0, r1=None):
            """(out_ap, in0_ap, in1_ap) for tile k's multiply, 2x-eligible."""
            Rk = (int(R[k]) if r1 is None else r1) - r0
            gi = int(group_of[k])
            roff = int(boffs[k]) - g_boff[gi] + r0  # round offset inside slab
            b0 = int(boffs[k]) + r0
            x_ap = (
                xs[gi % 3][:, roff : roff + Rk, :]
                .rearrange("p r (a b) -> p r a b", b=VDUP)
            )
            s_ap = (
                sc[gi % 2][:, roff : roff + Rk, :]
                .rearrange("p r (a b) -> p r a b", b=VDUP)
            )
            v_ap = (
                vex[:, b0 : b0 + Rk, :]
                .unsqueeze(2)
                .to_broadcast([P, Rk, F // VDUP, VDUP])
            )
            return s_ap, x_ap, v_ap

        with nc.Block() as block:

            @block.sync
            def _(sp):
                # first half-slab ahead of the consts: the bulk stream starts
                # at t=0 while nothing can consume it before ~3us anyway
                ra0, rb0 = g_halves[0][0]
                nc.sync.dma_start(
                    out=xs[0][:, ra0:rb0, :], in_=XRT[:, ra0:rb0, :]
                ).then_inc(s_xs[0], 16)
                nc.sync.dma_start(out=ident.ap(), in_=IDP.ap()).then_inc(s_cst, 16)
                nc.sync.dma_start(out=wsb.ap(), in_=WP.ap()).then_inc(s_cst, 16)
                nc.sync.dma_start(out=bias.ap(), in_=BIASP.ap()).then_inc(s_cst, 16)

                for gi, (k0, gs) in enumerate(groups):
                    if gi >= 3:
                        # xs slab reuse: all scale ops of group gi-3 done
                        klast = groups[gi - 3][0] + groups[gi - 3][1] - 1
                        for e in ("v", "p"):
                            sp.wait_ge(s_sem[e], int(cnt_after[e][klast]))
                    for hi, (ra, rb) in enumerate(g_halves[gi]):
                        if gi == 0 and hi == 0:
                            continue  # pre-issued above
                        nc.sync.dma_start(
                            out=xs[gi % 3][:, ra:rb, :],
                            in_=XRT[:, g_boff[gi] + ra : g_boff[gi] + rb, :],
                        ).then_inc(s_xs[2 * (gi % 3) + hi], 16)
                for i in range(6):
                    sp.wait_ge(s_xs[i], 16 * slot_loads[i])

            @block.vector
            def _(dve):
                dve.wait_ge(s_cst, 64)
                for k in range(NT):
                    if eng_of[k] != "v":
                        continue
                    gi = int(group_of[k])
                    if k == k_split:
                        klast = groups[gi - 2][0] + groups[gi - 2][1] - 1
                        for part, (r0, r1) in enumerate([(0, Ra), (Ra, int(R[k]))]):
                            si, cnt = split_xs[part]
                            dve.wait_ge(s_xs[si], 16 * cnt)
                            if part == 0 and gi >= 2:
                                dve.wait_ge(s_peA, klast + 1)
                            s_ap, x_ap, v_ap = scale_in_aps(k, r0, r1)
                            nc.vector.tensor_tensor(
                                out=s_ap, in0=x_ap, in1=v_ap,
                                op=mybir.AluOpType.mult,
                            ).then_inc(s_scv, 1)
                        continue
                    si, cnt = tile_xs_sem[k]
                    dve.wait_ge(s_xs[si], 16 * cnt)
                    if gi >= 2:
                        klast = groups[gi - 2][0] + groups[gi - 2][1] - 1
                        dve.wait_ge(s_peA, klast + 1)  # sc slab reuse
                    s_ap, x_ap, v_ap = scale_in_aps(k)
                    nc.vector.tensor_tensor(
                        out=s_ap, in0=x_ap, in1=v_ap, op=mybir.AluOpType.mult
                    ).then_inc(s_scv, 1)

            @block.gpsimd
            def _(pl):
                pl.wait_ge(s_cst, 64)
                for k in range(NT):
                    if eng_of[k] != "p":
                        continue
                    gi = int(group_of[k])
                    si, cnt = tile_xs_sem[k]
                    pl.wait_ge(s_xs[si], 16 * cnt)
                    if gi >= 2:
                        klast = groups[gi - 2][0] + groups[gi - 2][1] - 1
                        pl.wait_ge(s_peA, klast + 1)
                    s_ap, x_ap, v_ap = scale_in_aps(k)
                    nc.gpsimd.tensor_tensor(
                        out=s_ap, in0=x_ap, in1=v_ap, op=mybir.AluOpType.mult
                    ).then_inc(s_scp, 1)

            @block.tensor
            def _(pe):
                pe.wait_ge(s_cst, 64)
                for k in range(NT):
                    Rk = int(R[k])
                    gi = int(group_of[k])
                    k0, gs = groups[gi]
                    roff = int(boffs[k]) - g_boff[gi]
                    e = eng_of[k]
                    if k == k_split:
                        nv = int(cnt_after["v"][NT - 2])  # ops before the split pair
                        for part, (r0, r1) in enumerate([(0, Ra), (Ra, Rk)]):
                            bank = (k + part) % 3
                            pe.wait_ge(s_scv, nv + part + 1)
                            pe.wait_ge(s_acth, k - 2 + part)  # bank free
                            for r in range(r0, r1):
                                mm = nc.tensor.matmul(
                                    out=pha[bank][:, :P],
                                    lhsT=sc[gi % 2][:, roff + r, :],
                                    rhs=ident.ap(),
                                    start=(r == r0),
                                    stop=(r == r1 - 1),
                                )
                            mm.then_inc(s_peA, 1)
                        pe.wait_ge(s_acth, k + 2)  # both part copies done
                        pe.wait_ge(s_acto, gi - 1)
                        nc.tensor.matmul(
                            out=phb[gi % 2][:, :P], lhsT=wsb.ap(),
                            rhs=ht[gi % 2][:, :P], start=True, stop=False,
                        )
                        nc.tensor.matmul(
                            out=phb[gi % 2][:, :P], lhsT=wsb.ap(),
                            rhs=ht[gi % 2][:, P : 2 * P], start=False, stop=True,
                        ).then_inc(s_peB, 1)
                        continue
                    pe.wait_ge(s_sem[e], int(cnt_after[e][k]))
                    if k >= 3:
                        pe.wait_ge(s_acth, k - 2)  # pha slot reuse
                    for r in range(Rk):
                        mm = nc.tensor.matmul(
                            out=pha[k % 3][:, :P],
                            lhsT=sc[gi % 2][:, roff + r, :],
                            rhs=ident.ap(),
                            start=(r == 0),
                            stop=(r == Rk - 1),
                        )
                    mm.then_inc(s_peA, 1)
                    if k == k0 + gs - 1:
                        pe.wait_ge(s_acth, k + 1)
                        if gi >= 2:
                            pe.wait_ge(s_acto, gi - 1)
                        nc.tensor.matmul(
                            out=phb[gi % 2][:, : gs * P],
                            lhsT=wsb.ap(),
                            rhs=ht[gi % 2][:, : gs * P],
                            start=True,
                            stop=True,
                        ).then_inc(s_peB, 1)

            @block.scalar
            def _(act):
                nc.scalar.dma_start(out=vex.ap(), in_=VEX.ap()).then_inc(s_cst, 16)
                act.wait_ge(s_cst, 64)
                for k in range(NT):
                    gi = int(group_of[k])
                    k0, gs = groups[gi]
                    j = k - k0
                    if k == k_split:
                        act.wait_ge(s_peB, gi - 1)  # ht slot reuse
                        for part in range(2):
                            act.wait_ge(s_peA, k + 1 + part)
                            nc.scalar.copy(
                                ht[gi % 2][:, part * P : (part + 1) * P],
                                pha[(k + part) % 3][:, :P],
                            ).then_inc(s_acth, 1)
                        act.wait_ge(s_peB, gi + 1)
                        act.wait_ge(s_odma[gi % 2], 16 * (gi // 2))
                        nc.scalar.add(
                            osb[gi % 2][:, :P], phb[gi % 2][:, :P], bias.ap()
                        ).then_inc(s_acto, 1)
                        nc.scalar.dma_start(
                            out=OUT[:, k0 * P : (k0 + 1) * P],
                            in_=osb[gi % 2][:, :P],
                        ).then_inc(s_odma[gi % 2], 16)
                        continue
                    if j == 0 and gi >= 2:
                        act.wait_ge(s_peB, gi - 1)  # ht slot reuse
                    act.wait_ge(s_peA, k + 1)
                    nc.scalar.copy(
                        ht[gi % 2][:, j * P : (j + 1) * P], pha[k % 3][:, :P]
                    ).then_inc(s_acth, 1)
                    if j == gs - 1:
                        act.wait_ge(s_peB, gi + 1)
                        if gi >= 2:
                            act.wait_ge(s_odma[gi % 2], 16 * (gi // 2))  # osb reuse
                        nc.scalar.add(
                            osb[gi % 2][:, : gs * P],
                            phb[gi % 2][:, : gs * P],
                            bias.ap(),
                        ).then_inc(s_acto, 1)
                        nc.scalar.dma_start(
                            out=OUT[:, k0 * P : (k0 + gs) * P],
                            in_=osb[gi % 2][:, : gs * P],
                        ).then_inc(s_odma[gi % 2], 16)
                for i in range(2):
                    act.wait_ge(s_odma[i], 16 * len(range(i, NG, 2)))

        for s in all_sems:
            nc.sync.sem_clear(s)
    return nc


def _prep(x, edge_row, edge_col, edge_val):
    """Host-side sharding/layout prep."""
    deg = np.bincount(edge_row, minlength=N_NODES)
    order = np.argsort(deg, kind="stable")            # node ids by degree asc
    pos = np.empty(N_NODES, dtype=np.int64)
    pos[order] = np.arange(N_NODES)

    degs_padded = np.zeros(NPOS, dtype=np.int64)
    degs_padded[:N_NODES] = deg[order]
    R = degs_padded.reshape(N_TILES, SPAN).max(axis=1)
    R = np.maximum(R, 1).astype(np.int64)
    boff = np.zeros(N_TILES, dtype=np.int64)
    boff[1:] = np.cumsum(R)[:-1]

    # per-edge placement
    p = pos[edge_row]
    c = p % N_CORES
    slot = p // N_CORES
    k = slot // P
    j = slot % P
    sort_idx = np.argsort(edge_row, kind="stable")
    sorted_rows = edge_row[sort_idx]
    ranks = np.arange(N_EDGES) - np.searchsorted(sorted_rows, sorted_rows)
    r = np.empty(N_EDGES, dtype=np.int64)
    r[sort_idx] = ranks
    b = boff[k] + r

    B = int(R.sum())
    x16 = x.astype(np.float16)
    XRT = np.zeros((N_CORES, P, B, F), dtype=np.float16)
    VAL = np.zeros((N_CORES, P, B), dtype=np.float16)
    XRT[c, j, b] = x16[edge_col]
    VAL[c, j, b] = edge_val.astype(np.float16)
    VEX = np.repeat(VAL[:, :, :, None], VDUP, axis=3)
    return R, XRT, VEX, order


def kernel(x, edge_row, edge_col, edge_val, weight, bias_param):
    import sys
    for pth in ("/opt/trn_rl_repo",):
        if pth not in sys.path:
            sys.path.insert(0, pth)
    from concourse.bass_utils import run_bass_kernel_spmd

    x = np.asarray(x, dtype=np.float32)
    edge_row = np.asarray(edge_row, dtype=np.int32)
    edge_col = np.asarray(edge_col, dtype=np.int32)
    edge_val = np.asarray(edge_val, dtype=np.float32)
    weight = np.asarray(weight, dtype=np.float32)
    bias_param = np.asarray(bias_param, dtype=np.float32)

    R, XRT, VEX, order = _prep(x, edge_row, edge_col, edge_val)

    key = tuple(R.tolist())
    if key not in _KERNEL_CACHE:
        _KERNEL_CACHE[key] = _build_nc(R)
    nc = _KERNEL_CACHE[key]

    w16 = weight.astype(np.float16)
    bias2d = bias_param.reshape(F, 1).astype(np.float32)
    id16 = np.eye(P, dtype=np.float16)

    in_maps = [
        {
            "xrt": XRT[cid],
            "vex": VEX[cid],
            "w": w16,
            "bias": bias2d,
            "ident": id16,
        }
        for cid in range(N_CORES)
    ]

    res = run_bass_kernel_spmd(nc, in_maps, core_ids=list(range(N_CORES)))

    out_full = np.empty((N_NODES, F), dtype=np.float32)
    for cid in range(N_CORES):
        outT = res.results[cid]["out"].astype(np.float32)   # [F, SLOTS]
        gpos = np.arange(SLOTS) * N_CORES + cid   # global positions
        valid = gpos < N_NODES
        out_full[order[gpos[valid]]] = outT.T[valid]
    return out_full


# revision 20
# speedup vs baseline: 1.0112x; 1.0112x over previous
"""GCNConv (X @ W sparse-aggregated) Trainium2 kernel, 8-core SPMD.

Math: out = segment_sum(edge_val * (X@W)[edge_col], edge_row) + bias
Reformulated via associativity:  out = H @ W + bias, where
    H = segment_sum(edge_val * X[edge_col], edge_row)          # [N, F]

Sharding: destination nodes are sorted by in-degree and dealt round-robin
across the 8 cores, so every core gets an identical per-tile "round"
structure (same compiled program on all cores).  The host pre-gathers
X[edge_col] into a round-major, partition-contiguous layout so the device
only does large sequential DMA; the device then:
  1. scales each gathered row by its edge value (DVE/GPSIMD multiply; the
     edge values are pre-replicated 8x on host so the innermost AP dim is
     unit-stride 2-byte -> DVE 2x perf mode)
  2. scatter-reduces rounds into H.T tiles with PE matmuls against an
     identity (PSUM accumulation: lhsT=scaled rows, rhs=I)
  3. computes out.T = W.T @ H.T with a second PE matmul, adds bias during
     the PSUM->SBUF copy (ACT), and streams out.T to HBM.
The host un-permutes/transposes the per-core outputs into the full result.

Raw Bass (no Tile framework): this walrus build allows only ONE attached
sync-wait per compute instruction; standalone wait_ge sequencer ops have no
such limit and the pipeline is static, so explicit counters work.
DMA completion semaphores are per-buffer-slot: a slot's wait target always
equals the total count of DMAs ever issued on that semaphore at wait time,
so partial-completion skew across the 16 SDMA engines cannot fire it early.
"""

import numpy as np

N_NODES = 50000
N_EDGES = 800000
F = 128
P = 128
N_CORES = 8
SPAN = P * N_CORES               # 1024 degree-sorted nodes per tile-span
N_TILES = (N_NODES + SPAN - 1) // SPAN      # 49
NPOS = N_TILES * SPAN            # 50176 padded positions
SLOTS = N_TILES * P              # 6272 node slots per core
VDUP = 4                         # host-side replication of edge values

_KERNEL_CACHE = {}


def _scale_engine(k):
    """Scale engine per tile. All on DVE: its 2x perf mode matches DMA and PE
    rates (~90ns per round each), and concurrent GPSIMD tensor ops contend
    for the same SBUF ports, slowing DVE to 1x."""
    return "v"


def _build_nc(R):
    from contextlib import ExitStack

    import concourse.bass as bass
    import concourse.mybir as mybir

    f16 = mybir.dt.float16
    f32 = mybir.dt.float32

    NT = N_TILES
    B = int(np.sum(R))
    boffs = np.zeros(NT, dtype=np.int64)
    boffs[1:] = np.cumsum(R)[:-1]

    # group structure: 4 tiles per group (one DMA slab + one N=512 GEMM2)
    groups = []  # (first_tile, gsize)
    kk = 0
    while kk < NT:
        gs = min(4, NT - kk)
        groups.append((kk, gs))
        kk += gs
    NG = len(groups)
    group_of = np.zeros(NT, dtype=np.int64)
    for gi, (k0, gs) in enumerate(groups):
        group_of[k0 : k0 + gs] = gi
    # rounds per group and max (for slab sizing)
    gR = [int(sum(R[k0 : k0 + gs])) for (k0, gs) in groups]
    GRmax = max(gR)
    g_boff = [int(boffs[k0]) for (k0, _gs) in groups]

    # split each group's slab load in two (by tiles) for finer pipelining
    g_halves = []          # per group: list of (round_start, round_end) in slab
    tile_xs_sem = {}       # tile -> (sem index, completed-load count on it)
    slot_loads = [0, 0, 0, 0, 0, 0]
    for gi, (k0, gs) in enumerate(groups):
        h1 = (gs + 1) // 2
        cut = int(boffs[k0 + h1 - 1] + R[k0 + h1 - 1] - g_boff[gi]) if h1 > 0 else 0
        halves = [(0, cut)]
        if cut < gR[gi]:
            halves.append((cut, gR[gi]))
        g_halves.append(halves)
        for hi, (ra, rb) in enumerate(halves):
            si = 2 * (gi % 3) + hi
            slot_loads[si] += 1
            lo = k0 if hi == 0 else k0 + h1
            hi_t = (k0 + h1 - 1) if hi == 0 else (k0 + gs - 1)
            for k in range(lo, hi_t + 1):
                tile_xs_sem[k] = (si, slot_loads[si])

    eng_of = [_scale_engine(k) for k in range(NT)]
    cnt_after = {"v": np.zeros(NT, dtype=np.int64), "p": np.zeros(NT, dtype=np.int64)}
    cv = cp = 0
    for k in range(NT):
        if eng_of[k] == "v":
            cv += 1
        else:
            cp += 1
        cnt_after["v"][k] = cv
        cnt_after["p"][k] = cp

    nc = bass.Bass(target_bir_lowering=False, debug=False)

    XRT = nc.declare_dram_parameter("xrt", [P, B, F], f16, isOutput=False)
    VEX = nc.declare_dram_parameter("vex", [P, B, VDUP], f16, isOutput=False)
    WP = nc.declare_dram_parameter("w", [F, F], f16, isOutput=False)
    BIASP = nc.declare_dram_parameter("bias", [F, 1], f32, isOutput=False)
    IDP = nc.declare_dram_parameter("ident", [P, P], f16, isOutput=False)
    OUT = nc.declare_dram_parameter("out", [F, SLOTS], f16, isOutput=True)

    with ExitStack() as ctx:
        ident = ctx.enter_context(nc.sbuf_tensor("identsb", [P, P], f16))
        wsb = ctx.enter_context(nc.sbuf_tensor("wsb", [F, F], f16))
        vex = ctx.enter_context(nc.sbuf_tensor("vexsb", [P, B, VDUP], f16))
        bias = ctx.enter_context(nc.sbuf_tensor("biassb", [F, 1], f32))
        xs = [ctx.enter_context(nc.sbuf_tensor(f"xs{i}", [P, GRmax, F], f16)) for i in range(3)]
        sc = [ctx.enter_context(nc.sbuf_tensor(f"sc{i}", [P, GRmax, F], f16)) for i in range(2)]
        ht = [ctx.enter_context(nc.sbuf_tensor(f"ht{i}", [P, 4 * P], f16)) for i in range(2)]
        osb = [ctx.enter_context(nc.sbuf_tensor(f"osb{i}", [P, 4 * P], f16)) for i in range(2)]
        pha = [ctx.enter_context(nc.psum_tensor(f"pha{i}", [P, 512], f32)) for i in range(3)]
        phb = [ctx.enter_context(nc.psum_tensor(f"phb{i}", [P, 512], f32)) for i in range(2)]
        phw = ctx.enter_context(nc.psum_tensor("phw", [P, 512], f32))

        s_cst = ctx.enter_context(nc.semaphore("s_cst"))
        s_xs = [ctx.enter_context(nc.semaphore(f"s_xs{i}")) for i in range(6)]
        s_scv = ctx.enter_context(nc.semaphore("s_scv"))
        s_scp = ctx.enter_context(nc.semaphore("s_scp"))
        s_peA = ctx.enter_context(nc.semaphore("s_peA"))
        s_peB = ctx.enter_context(nc.semaphore("s_peB"))
        s_acth = ctx.enter_context(nc.semaphore("s_acth"))
        s_acto = ctx.enter_context(nc.semaphore("s_acto"))
        s_odma = [ctx.enter_context(nc.semaphore(f"s_odma{i}")) for i in range(2)]
        all_sems = [s_cst, *s_xs, s_scv, s_scp, s_peA, s_peB, s_acth, s_acto, *s_odma]
        s_sem = {"v": s_scv, "p": s_scp}

        for s in all_sems:
            nc.sync.sem_clear(s)
        nc.all_engine_barrier()

        def scale_in_aps(k):
            """(out_ap, in0_ap, in1_ap) for tile k's multiply, 2x-eligible."""
            Rk = int(R[k])
            gi = int(group_of[k])
            roff = int(boffs[k]) - g_boff[gi]      # round offset inside slab
            b0 = int(boffs[k])
            x_ap = (
                xs[gi % 3][:, roff : roff + Rk, :]
                .rearrange("p r (a b) -> p r a b", b=VDUP)
            )
            s_ap = (
                sc[gi % 2][:, roff : roff + Rk, :]
                .rearrange("p r (a b) -> p r a b", b=VDUP)
            )
            v_ap = (
                vex[:, b0 : b0 + Rk, :]
                .unsqueeze(2)
                .to_broadcast([P, Rk, F // VDUP, VDUP])
            )
            return s_ap, x_ap, v_ap

        with nc.Block() as block:

            @block.sync
            def _(sp):
                # first half-slab ahead of the consts: the bulk stream starts
                # at t=0 while nothing can consume it before ~3us anyway
                ra0, rb0 = g_halves[0][0]
                nc.sync.dma_start(
                    out=xs[0][:, ra0:rb0, :], in_=XRT[:, ra0:rb0, :]
                ).then_inc(s_xs[0], 16)
                nc.sync.dma_start(out=ident.ap(), in_=IDP.ap()).then_inc(s_cst, 16)
                nc.sync.dma_start(out=wsb.ap(), in_=WP.ap()).then_inc(s_cst, 16)
                nc.sync.dma_start(out=bias.ap(), in_=BIASP.ap()).then_inc(s_cst, 16)

                for gi, (k0, gs) in enumerate(groups):
                    if gi >= 3:
                        # xs slab reuse: all scale ops of group gi-3 done
                        klast = groups[gi - 3][0] + groups[gi - 3][1] - 1
                        for e in ("v", "p"):
                            sp.wait_ge(s_sem[e], int(cnt_after[e][klast]))
                    for hi, (ra, rb) in enumerate(g_halves[gi]):
                        if gi == 0 and hi == 0:
                            continue  # pre-issued above
                        nc.sync.dma_start(
                            out=xs[gi % 3][:, ra:rb, :],
                            in_=XRT[:, g_boff[gi] + ra : g_boff[gi] + rb, :],
                        ).then_inc(s_xs[2 * (gi % 3) + hi], 16)
                for i in range(6):
                    sp.wait_ge(s_xs[i], 16 * slot_loads[i])

            @block.vector
            def _(dve):
                dve.wait_ge(s_cst, 64)
                for k in range(NT):
                    if eng_of[k] != "v":
                        continue
                    gi = int(group_of[k])
                    si, cnt = tile_xs_sem[k]
                    dve.wait_ge(s_xs[si], 16 * cnt)
                    if gi >= 2:
                        klast = groups[gi - 2][0] + groups[gi - 2][1] - 1
                        dve.wait_ge(s_peA, klast + 1)  # sc slab reuse
                    s_ap, x_ap, v_ap = scale_in_aps(k)
                    nc.vector.tensor_tensor(
                        out=s_ap, in0=x_ap, in1=v_ap, op=mybir.AluOpType.mult
                    ).then_inc(s_scv, 1)

            @block.gpsimd
            def _(pl):
                pl.wait_ge(s_cst, 64)
                for k in range(NT):
                    if eng_of[k] != "p":
                        continue
                    gi = int(group_of[k])
                    si, cnt = tile_xs_sem[k]
                    pl.wait_ge(s_xs[si], 16 * cnt)
                    if gi >= 2:
                        klast = groups[gi - 2][0] + groups[gi - 2][1] - 1
                        pl.wait_ge(s_peA, klast + 1)
                    s_ap, x_ap, v_ap = scale_in_aps(k)
                    nc.gpsimd.tensor_tensor(
                        out=s_ap, in0=x_ap, in1=v_ap, op=mybir.AluOpType.mult
                    ).then_inc(s_scp, 1)

            @block.tensor
            def _(pe):
                pe.wait_ge(s_cst, 64)
                for k in range(NT):
                    Rk = int(R[k])
                    gi = int(group_of[k])
                    k0, gs = groups[gi]
                    roff = int(boffs[k]) - g_boff[gi]
                    e = eng_of[k]
                    pe.wait_ge(s_sem[e], int(cnt_after[e][k]))
                    if k >= 3:
                        pe.wait_ge(s_acth, k - 2)  # pha slot reuse
                    for r in range(Rk):
                        mm = nc.tensor.matmul(
                            out=pha[k % 3][:, :P],
                            lhsT=sc[gi % 2][:, roff + r, :],
                            rhs=ident.ap(),
                            start=(r == 0),
                            stop=(r == Rk - 1),
                        )
                    mm.then_inc(s_peA, 1)
                    if k == k0 + gs - 1:
                        pe.wait_ge(s_acth, k + 1)
                        if gi >= 2:
                            pe.wait_ge(s_acto, gi - 1)
                        nc.tensor.matmul(
                            out=phb[gi % 2][:, : gs * P],
                            lhsT=wsb.ap(),
                            rhs=ht[gi % 2][:, : gs * P],
                            start=True,
                            stop=True,
                        ).then_inc(s_peB, 1)

            @block.scalar
            def _(act):
                nc.scalar.dma_start(out=vex.ap(), in_=VEX.ap()).then_inc(s_cst, 16)
                act.wait_ge(s_cst, 64)
                for k in range(NT):
                    gi = int(group_of[k])
                    k0, gs = groups[gi]
                    j = k - k0
                    if j == 0 and gi >= 2:
                        act.wait_ge(s_peB, gi - 1)  # ht slot reuse
                    act.wait_ge(s_peA, k + 1)
                    nc.scalar.copy(
                        ht[gi % 2][:, j * P : (j + 1) * P], pha[k % 3][:, :P]
                    ).then_inc(s_acth, 1)
                    if j == gs - 1:
                        act.wait_ge(s_peB, gi + 1)
                        if gi >= 2:
                            act.wait_ge(s_odma[gi % 2], 16 * (gi // 2))  # osb reuse
                        nc.scalar.add(
                            osb[gi % 2][:, : gs * P],
                            phb[gi % 2][:, : gs * P],
                            bias.ap(),
                        ).then_inc(s_acto, 1)
                        nc.scalar.dma_start(
                            out=OUT[:, k0 * P : (k0 + gs) * P],
                            in_=osb[gi % 2][:, : gs * P],
                        ).then_inc(s_odma[gi % 2], 16)
                for i in range(2):
                    act.wait_ge(s_odma[i], 16 * len(range(i, NG, 2)))

        for s in all_sems:
            nc.sync.sem_clear(s)
    return nc


def _prep(x, edge_row, edge_col, edge_val):
    """Host-side sharding/layout prep."""
    deg = np.bincount(edge_row, minlength=N_NODES)
    order = np.argsort(deg, kind="stable")            # node ids by degree asc
    pos = np.empty(N_NODES, dtype=np.int64)
    pos[order] = np.arange(N_NODES)

    degs_padded = np.zeros(NPOS, dtype=np.int64)
    degs_padded[:N_NODES] = deg[order]
    R = degs_padded.reshape(N_TILES, SPAN).max(axis=1)
    R = np.maximum(R, 1).astype(np.int64)
    boff = np.zeros(N_TILES, dtype=np.int64)
    boff[1:] = np.cumsum(R)[:-1]

    # per-edge placement
    p = pos[edge_row]
    c = p % N_CORES
    slot = p // N_CORES
    k = slot // P
    j = slot % P
    sort_idx = np.argsort(edge_row, kind="stable")
    sorted_rows = edge_row[sort_idx]
    ranks = np.arange(N_EDGES) - np.searchsorted(sorted_rows, sorted_rows)
    r = np.empty(N_EDGES, dtype=np.int64)
    r[sort_idx] = ranks
    b = boff[k] + r

    B = int(R.sum())
    x16 = x.astype(np.float16)
    XRT = np.zeros((N_CORES, P, B, F), dtype=np.float16)
    VAL = np.zeros((N_CORES, P, B), dtype=np.float16)
    XRT[c, j, b] = x16[edge_col]
    VAL[c, j, b] = edge_val.astype(np.float16)
    VEX = np.repeat(VAL[:, :, :, None], VDUP, axis=3)
    return R, XRT, VEX, order


def kernel(x, edge_row, edge_col, edge_val, weight, bias_param):
    import sys
    for pth in ("/opt/trn_rl_repo",):
        if pth not in sys.path:
            sys.path.insert(0, pth)
    from concourse.bass_utils import run_bass_kernel_spmd

    x = np.asarray(x, dtype=np.float32)
    edge_row = np.asarray(edge_row, dtype=np.int32)
    edge_col = np.asarray(edge_col, dtype=np.int32)
    edge_val = np.asarray(edge_val, dtype=np.float32)
    weight = np.asarray(weight, dtype=np.float32)
    bias_param = np.asarray(bias_param, dtype=np.float32)

    R, XRT, VEX, order = _prep(x, edge_row, edge_col, edge_val)

    key = tuple(R.tolist())
    if key not in _KERNEL_CACHE:
        _KERNEL_CACHE[key] = _build_nc(R)
    nc = _KERNEL_CACHE[key]

    w16 = weight.astype(np.float16)
    bias2d = bias_param.reshape(F, 1).astype(np.float32)
    id16 = np.eye(P, dtype=np.float16)

    in_maps = [
        {
            "xrt": XRT[cid],
            "vex": VEX[cid],
            "w": w16,
            "bias": bias2d,
            "ident": id16,
        }
        for cid in range(N_CORES)
    ]

    res = run_bass_kernel_spmd(nc, in_maps, core_ids=list(range(N_CORES)))

    out_full = np.empty((N_NODES, F), dtype=np.float32)
    for cid in range(N_CORES):
        outT = res.results[cid]["out"].astype(np.float32)   # [F, SLOTS]
        gpos = np.arange(SLOTS) * N_CORES + cid   # global positions
        valid = gpos < N_NODES
        out_full[order[gpos[valid]]] = outT.T[valid]
    return out_full
